# revision 1
# baseline (speedup 1.0000x reference)
"""Banded local attention (ATTN_WIDTH=128) with exp-before-softmax, on 8 trn2 cores.

Reference math (per batch b, row q, full S=4096 columns):
    s      = Q K^T / 8
    a      = exp(s - rowmax(s))          # exact full-row max m1 required
    a_mask = a * band_mask               # keep j - i in [-64, 63]
    w      = softmax(a_mask)             # over all 4096 entries incl. zeros
    out    = w V

Reformulation (validated vs reference):
  - a_mask in [0, 1] so the second softmax needs no max shift:
        w_k = e^{a_k} / (sum_band e^{a_j} + (S - nb))
  - 256-wide window per 128-row q-tile, multiplicative 0/1 mask M:
        eg    = exp(exp(sw - m1) * M)      # masked lanes -> exp(0) = 1
        denom = sum_w eg + (S - 256)
        numer = eg @ V_win + (sum_all V - sum_win V)
        out   = numer / denom              # band-count nb cancels

Sharding: 8 cores = 4 batches x 2 query-halves of 2048 rows.

The scores are computed ONCE per q-tile as 4 PSUM pairs of [128, 1024]
(8 x 512-col float32r matmuls). K columns are reordered per core so that
the window of q-tile i sits at compile-time columns [128i, 128i+256)
regardless of the core's query offset (SPMD-uniform):
  cols [0, 2176)    = padded window slice (pads filled with real columns
                      borrowed from the tail so every k appears exactly once)
  cols [2176, 4096) = remaining columns (row max only)
The row max is order-invariant, so the reorder is free.

Engine split per q-tile:
  PE : 8 m1/window matmuls (f32r, 1 col/cycle), 2 eg transposes, 2 eg@V
  DVE: 4 pair reduce_max + negated combine, denom+recip, numer+CV add,
       final 1/denom scale
  ACT: exp1 (bias -m1, reads the window straight from score PSUM),
       exp2 (fused row-sum accum), eg->f32r cast copy
  GP : window mask multiply
"""

import sys

if "/opt/trn_rl_repo" not in sys.path:
    sys.path.insert(0, "/opt/trn_rl_repo")

from contextlib import ExitStack

import numpy as np

import concourse.bacc as bacc
import concourse.bass as bass
import concourse.tile as tile
from concourse import mybir
from concourse.bass_utils import run_bass_kernel_spmd

B, S, D = 4, 4096, 64
ATTN_WIDTH = 128
PAD = ATTN_WIDTH // 2          # 64
W = 2 * ATTN_WIDTH             # 256 window per q-tile
HALF = S // 2                  # 2048 rows per core
NT = HALF // 128               # 16 q-tiles per core
KSLICE = HALF + 2 * PAD        # 2176 window-slice columns
N_CORES = 8
F32 = mybir.dt.float32
F32R = mybir.dt.float32r
BF16 = mybir.dt.bfloat16

_CACHE = {}


def _round_f32r(x: np.ndarray) -> np.ndarray:
    """Round fp32 to float32r (11-bit mantissa, round-to-nearest) like walrus."""
    u = np.ascontiguousarray(x, dtype=np.float32).view(np.uint32)
    r = ((u.astype(np.uint64) + 0x800) & 0xFFFFF000).astype(np.uint32)
    return r.view(np.float32)


def _emit(ctx: ExitStack, tc, params):
    nc = tc.nc
    Exp = mybir.ActivationFunctionType.Exp
    mx = mybir.AluOpType.max

    const = ctx.enter_context(tc.tile_pool(name="const", bufs=1))
    work = ctx.enter_context(tc.tile_pool(name="work", bufs=2))
    outp = ctx.enter_context(tc.tile_pool(name="outp", bufs=3))
    ps_sc = ctx.enter_context(tc.tile_pool(name="ps_sc", bufs=3, space="PSUM"))
    ps_fv = ctx.enter_context(tc.tile_pool(name="ps_fv", bufs=2, space="PSUM"))

    qtr_s = const.tile([64, HALF], F32R)
    ktr_s = const.tile([64, S], F32R)
    vsr_s = const.tile([128, (NT + 1) * 64], F32R)
    ma_s = const.tile([128, 3 * W], F32)
    cvb_s = const.tile([128, NT * 64], F32)
    id_s = const.tile([128, 128], F32R)
    # issue order = first-use order: tile 0 needs qtr[:, :128] + all 4 ktr pairs
    nc.sync.dma_start(qtr_s[:, 0:128], params["qtr"][:, 0:128])
    for c in range(8):
        nc.sync.dma_start(
            ktr_s[:, 512 * c : 512 * (c + 1)],
            params["ktr"][:, 512 * c : 512 * (c + 1)],
        )
    nc.sync.dma_start(ma_s[:], params["ma"][:])
    nc.sync.dma_start(vsr_s[:, 0 : 8 * 64], params["vsr"][:, 0 : 8 * 64])
    nc.sync.dma_start(id_s[:], params["idf"][:])
    for c in range(3):
        nc.sync.dma_start(
            qtr_s[:, 128 + 640 * c : 128 + 640 * (c + 1)],
            params["qtr"][:, 128 + 640 * c : 128 + 640 * (c + 1)],
        )
    nc.sync.dma_start(
        vsr_s[:, 8 * 64 : (NT + 1) * 64], params["vsr"][:, 8 * 64 : (NT + 1) * 64]
    )
    nc.sync.dma_start(cvb_s[:], params["cvb"][:])
    out = params["out"]

    for i in range(NT):
        qtile_r = qtr_s[:, 128 * i : 128 * (i + 1)]
        wp = (128 * i) // 1024          # pair holding the window start
        lo = 128 * i - 1024 * wp        # window offset within pair wp
        cross = lo + W > 1024           # window spans pairs wp, wp+1
        order = [wp] + ([wp + 1] if cross else [])
        order += [p for p in range(4) if p not in order]

        pair_tiles = {}
        mp = work.tile([128, 4], F32, tag="mp", bufs=3)
        for j, p in enumerate(order):
            sc = ps_sc.tile([128, 1024], F32, tag="sc")
            pair_tiles[p] = sc
            base = 1024 * p
            nc.tensor.matmul(
                sc[:, 0:512], qtile_r, ktr_s[:, base : base + 512],
                start=True, stop=True,
            )
            nc.tensor.matmul(
                sc[:, 512:1024], qtile_r, ktr_s[:, base + 512 : base + 1024],
                start=True, stop=True,
            )
            nc.vector.reduce_max(
                mp[:, j : j + 1], sc[:], axis=mybir.AxisListType.X
            )
        nm1 = work.tile([128, 1], F32, tag="nm1")
        nc.vector.tensor_reduce(
            nm1[:], mp[:], axis=mybir.AxisListType.X, op=mx, negate=True
        )

        # --- ew = exp(window) off the critical chain (no m1 dependency);
        #     masked on GPSIMD early; m1 applied later as exp(-m1) scale ---
        ew = work.tile([128, W], F32, tag="ew", bufs=3)
        if not cross:
            nc.scalar.activation(ew[:], pair_tiles[wp][:, lo : lo + W], Exp)
        else:
            n1 = 1024 - lo
            nc.scalar.activation(ew[:, 0:n1], pair_tiles[wp][:, lo:1024], Exp)
            nc.scalar.activation(
                ew[:, n1:W], pair_tiles[wp + 1][:, 0 : W - n1], Exp
            )
        msel = 0 if i == 0 else (2 if i == NT - 1 else 1)
        ewm = work.tile([128, W], F32, tag="ewm", bufs=3)
        nc.gpsimd.tensor_tensor(
            out=ewm[:], in0=ew[:], in1=ma_s[:, W * msel : W * (msel + 1)],
            op=mybir.AluOpType.mult,
        )

        em1 = work.tile([128, 1], F32, tag="em1")
        nc.scalar.activation(em1[:], nm1[:], Exp)
        am = work.tile([128, W], F32, tag="am", bufs=3)
        nc.vector.tensor_scalar_mul(am[:], ewm[:], em1[:])
        eg = work.tile([128, W], F32R, tag="eg", bufs=3)
        seg = work.tile([128, 1], F32, tag="seg")
        nc.scalar.activation(eg[:], am[:], Exp, accum_out=seg[:])

        # denom = seg + (S - W);  rec = 1 / denom
        den = work.tile([128, 1], F32, tag="den")
        nc.gpsimd.tensor_scalar_add(den[:], seg[:], float(S - W))
        rec = work.tile([128, 1], F32, tag="rec")
        nc.vector.reciprocal(rec[:], den[:])

        # --- numer = eg @ V_win + CV_i (eg transposed on PE) ---
        fvt = ps_fv.tile([128, 320], F32, tag="fvt")
        egt_ps = fvt[:, 0:256].bitcast(F32R)
        numer = fvt[:, 256:320]
        nc.tensor.transpose(egt_ps[:, 0:128], eg[:, 0:128], id_s[:])
        nc.tensor.transpose(egt_ps[:, 128:256], eg[:, 128:256], id_s[:])
        egt = work.tile([128, W], F32R, tag="egt_sb", bufs=3)
        nc.scalar.copy(egt[:], egt_ps[:])
        nc.tensor.matmul(
            numer[:], egt[:, 0:128], vsr_s[:, 64 * i : 64 * i + 64],
            start=True, stop=False,
        )
        nc.tensor.matmul(
            numer[:], egt[:, 128:256], vsr_s[:, 64 * (i + 1) : 64 * (i + 1) + 64],
            start=False, stop=True,
        )

        ncv = work.tile([128, 64], F32, tag="ncv")
        nc.vector.tensor_tensor(
            out=ncv[:], in0=numer[:], in1=cvb_s[:, 64 * i : 64 * i + 64],
            op=mybir.AluOpType.add,
        )
        out_sb = outp.tile([128, 64], F32, tag="out_sb")
        nc.scalar.activation(
            out_sb[:], ncv[:], mybir.ActivationFunctionType.Copy,
            bias=0.0, scale=rec[:],
        )
        nc.sync.dma_start(out[128 * i : 128 * (i + 1), :], out_sb[:])


def build_program():
    nc = bacc.Bacc("TRN2", target_bir_lowering=False, debug=False)
    params = {
        "qtr": nc.declare_dram_parameter("qtr", [64, HALF], F32R, isOutput=False),
        "ktr": nc.declare_dram_parameter("ktr", [64, S], F32R, isOutput=False),
        "vsr": nc.declare_dram_parameter(
            "vsr", [128, (NT + 1) * 64], F32R, isOutput=False
        ),
        "ma": nc.declare_dram_parameter("ma", [128, 3 * W], F32, isOutput=False),
        "cvb": nc.declare_dram_parameter("cvb", [128, NT * 64], F32, isOutput=False),
        "idf": nc.declare_dram_parameter("idf", [128, 128], F32R, isOutput=False),
        "out": nc.declare_dram_parameter("out", [HALF, D], F32, isOutput=True),
    }
    with tile.TileContext(nc) as tc:
        with ExitStack() as ctx:
            _emit(ctx, tc, params)
    nc.compile()
    return nc


def make_in_maps(Q, K, V):
    """Full inputs -> list of 8 per-core input dicts."""
    Q = np.ascontiguousarray(np.asarray(Q, dtype=np.float32))
    K = np.ascontiguousarray(np.asarray(K, dtype=np.float32))
    V = np.ascontiguousarray(np.asarray(V, dtype=np.float32))

    idf = np.eye(128, dtype=np.float32)
    r = np.arange(128)[:, None]
    c = np.arange(W)[None, :]
    base_band = (c >= r) & (c < r + 128)

    in_maps = []
    for core in range(N_CORES):
        b, h = divmod(core, 2)
        off = h * HALF
        # fold the 1/sqrt(D) = 1/8 score scale into Q (exact: power of two)
        qt = np.ascontiguousarray(Q[b, off : off + HALF].T) * np.float32(0.125)
        qtr = _round_f32r(qt)

        # K column order: [window slice (pads borrowed from elsewhere) | rest]
        if h == 0:
            order = np.concatenate(
                [np.arange(2112, 2176), np.arange(0, 2112), np.arange(2176, S)]
            )
        else:
            order = np.concatenate(
                [np.arange(1984, S), np.arange(1920, 1984), np.arange(0, 1920)]
            )
        ktr = _round_f32r(np.ascontiguousarray(K[b].T[:, order]))

        Vpad = np.zeros((S + 2 * PAD, D), dtype=np.float32)
        Vpad[PAD : PAD + S] = V[b]
        vsl = Vpad[off : off + KSLICE]                                # [2176, 64]
        vsl_r = _round_f32r(vsl)
        vsr = np.ascontiguousarray(
            vsl_r.reshape(NT + 1, 128, D).transpose(1, 0, 2).reshape(
                128, (NT + 1) * 64
            )
        )

        # multiplicative masks: [tile0 | interior | tile15], each [128, 256]
        interior = base_band.astype(np.float32)
        m0 = interior
        m15 = interior
        if h == 0:  # global q-tile 0: need k >= 0  -> c >= 64
            m0 = (base_band & (c >= PAD)).astype(np.float32)
        else:  # global last tile: k < S -> c < 192
            m15 = (base_band & (c < 192)).astype(np.float32)
        ma = np.ascontiguousarray(np.concatenate([m0, interior, m15], axis=1))

        # CV_i = sum_all V (exact) - sum_window V_rounded, broadcast to rows
        sv = V[b].sum(axis=0, dtype=np.float32)
        cv = np.zeros((NT, 64), dtype=np.float32)
        for i in range(NT):
            cv[i] = sv - vsl_r[128 * i : 128 * i + W].sum(axis=0, dtype=np.float32)
        cvb = np.ascontiguousarray(
            np.broadcast_to(cv.reshape(1, NT * 64), (128, NT * 64))
        ).astype(np.float32)

        in_maps.append(
            {"qtr": qtr, "ktr": ktr, "vsr": vsr, "ma": ma, "cvb": cvb, "idf": idf}
        )
    return in_maps


def _get_program():
    if "nc" not in _CACHE:
        _CACHE["nc"] = build_program()
    return _CACHE["nc"]


def kernel(Q, K, V):
    nc = _get_program()
    in_maps = make_in_maps(Q, K, V)
    res = run_bass_kernel_spmd(nc, in_maps, list(range(N_CORES)))
    out = np.zeros((B, S, D), dtype=np.float32)
    for core in range(N_CORES):
        b, h = divmod(core, 2)
        out[b, h * HALF : (h + 1) * HALF] = res.results[core]["out"]
    return out

